# revision 2
# baseline (speedup 1.0000x reference)
"""Trainium2 Bass kernel for nn_AFM (attentional factorization machine).

Mathematical reduction used (validated against the reference):
  - softmax over a size-1 axis == 1, so the attention MLP is dead code and
    fAtt = mean(fPI, axis=1).
  - FM identity per (b, m): sum_{i<j} x_i x_j = ((sum_i x_i)^2 - sum_i x_i^2)/2
    with x_i = dense[b,i,m] * v[i,m].
  So with S1[b,m] = sum_n dense[b,n,m] v[n,m], S2[b,m] = sum_n (dense[b,n,m] v[n,m])^2,
  c[m] = Wp[m] / (2 * P):
    out[b] = sum_n dense[b,n,0] Wl[n] + bl + bp + sum_m c[m] (S1[b,m]^2 - S2[b,m])

Sharding: pure data parallel, batch 4096 -> 512 rows on each of 8 cores.
"""

import numpy as np

B, N, M = 4096, 32, 64
NM = N * M                  # 2048
NCORES = 8
BS = B // NCORES            # 512 rows per core
TILES = BS // 128           # 4 tiles of 128 batch rows per core
P_PAIRS = N * (N - 1) // 2  # 496

_CACHE = {}


def _build_program():
    import concourse.tile as tile
    from concourse import bacc, mybir

    f32 = mybir.dt.float32
    alu = mybir.AluOpType

    nc = bacc.Bacc("TRN2", target_bir_lowering=False, debug=False)
    dense = nc.dram_tensor("dense", [BS, NM], f32, kind="ExternalInput").ap()
    vrep = nc.dram_tensor("vrep", [128, NM], f32, kind="ExternalInput").ap()
    crep = nc.dram_tensor("crep", [128, M], f32, kind="ExternalInput").ap()
    wlrep = nc.dram_tensor("wlrep", [128, N], f32, kind="ExternalInput").ap()
    cst = nc.dram_tensor("cst", [128, 1], f32, kind="ExternalInput").ap()
    out = nc.dram_tensor("out", [BS], f32, kind="ExternalOutput").ap()

    with tile.TileContext(nc) as tc:
        with (
            tc.tile_pool(name="params", bufs=1) as ppool,
            tc.tile_pool(name="data", bufs=2) as dpool,
            tc.tile_pool(name="work", bufs=2) as wpool,
            tc.tile_pool(name="small", bufs=2) as spool,
        ):
            vrep_t = ppool.tile([128, NM], f32, tag="vrep")
            nc.sync.dma_start(vrep_t[:], vrep[:, :])
            crep_t = ppool.tile([128, M], f32, tag="crep")
            nc.sync.dma_start(crep_t[:], crep[:, :])
            wlrep_t = ppool.tile([128, N], f32, tag="wlrep")
            nc.sync.dma_start(wlrep_t[:], wlrep[:, :])
            cst_t = ppool.tile([128, 1], f32, tag="cst")
            nc.sync.dma_start(cst_t[:], cst[:, :])

            for t in range(TILES):
                d = dpool.tile([128, NM], f32, tag="d")
                nc.sync.dma_start(d[:], dense[128 * t : 128 * (t + 1), :])

                dv = wpool.tile([128, NM], f32, tag="dv")
                nc.vector.tensor_mul(dv[:], d[:], vrep_t[:])

                # reduce over n (stride M in the flat (n,m) axis), keep m
                dv_v = dv[:].rearrange("p (n m) -> p m n", n=N)
                s1 = spool.tile([128, M], f32, tag="s1")
                nc.vector.tensor_reduce(
                    s1[:], dv_v, axis=mybir.AxisListType.X, op=alu.add
                )

                sq = wpool.tile([128, NM], f32, tag="sq")
                nc.scalar.square(sq[:], dv[:])
                sq_v = sq[:].rearrange("p (n m) -> p m n", n=N)
                s2 = spool.tile([128, M], f32, tag="s2")
                nc.vector.tensor_reduce(
                    s2[:], sq_v, axis=mybir.AxisListType.X, op=alu.add
                )

                # chained fused reduces (custom-DVE TTR: accum = s0 + sum(in0*in1*s1)):
                #   pc1 = (bl+bp) + sum_m c*S1^2
                #   pc2 = pc1 - sum_m c*S2
                #   o2  = pc2 + sum_n dense[:,n,0]*Wl[n]
                from concourse.dve_ops import TENSOR_TENSOR_REDUCE as CTTR

                cs1 = spool.tile([128, M], f32, tag="cs1")
                nc.vector.tensor_mul(cs1[:], s1[:], crep_t[:])
                junk_a = spool.tile([128, M], f32, tag="junk_a")
                pc1 = spool.tile([128, 1], f32, tag="pc1")
                nc.vector._custom_dve(
                    CTTR, out=junk_a[:], in0=cs1[:], in1=s1[:],
                    s0=cst_t[:], s1=1.0, accum_out=pc1[:],
                )
                junk_b = spool.tile([128, M], f32, tag="junk_b")
                pc2 = spool.tile([128, 1], f32, tag="pc2")
                nc.vector._custom_dve(
                    CTTR, out=junk_b[:], in0=s2[:], in1=crep_t[:],
                    s0=pc1[:], s1=-1.0, accum_out=pc2[:],
                )

                d_col0 = (
                    d[:]
                    .rearrange("p (n m) -> p n m", n=N)[:, :, 0:1]
                    .rearrange("p n one -> p (n one)")
                )
                junk_c = spool.tile([128, N], f32, tag="junk_c")
                o2 = spool.tile([128, 1], f32, tag="o2")
                nc.vector._custom_dve(
                    CTTR, out=junk_c[:], in0=d_col0, in1=wlrep_t[:],
                    s0=pc2[:], s1=1.0, accum_out=o2[:],
                )

                nc.sync.dma_start(out[128 * t : 128 * (t + 1)], o2[:])

    nc.compile()
    return nc


def _get_program():
    if "nc" not in _CACHE:
        _CACHE["nc"] = _build_program()
    return _CACHE["nc"]


def _host_prep(inputs):
    dense = np.ascontiguousarray(
        np.asarray(inputs["dense"], dtype=np.float32).reshape(B, NM)
    )
    v = np.asarray(inputs["v"], dtype=np.float32).reshape(1, NM)
    Wl = np.asarray(inputs["Wl"], dtype=np.float32).reshape(N)
    Wp = np.asarray(inputs["Wp"], dtype=np.float32).reshape(M)
    bl = float(np.asarray(inputs["bl"], dtype=np.float32).reshape(-1)[0])
    bp = float(np.asarray(inputs["bp"], dtype=np.float32).reshape(-1)[0])

    c = (Wp / (2.0 * P_PAIRS)).astype(np.float32)
    vrep = np.ascontiguousarray(np.broadcast_to(v, (128, NM)))
    crep = np.ascontiguousarray(np.broadcast_to(c[None, :], (128, M)))
    wlrep = np.ascontiguousarray(np.broadcast_to(Wl[None, :], (128, N)))
    cst = np.full((128, 1), bl + bp, dtype=np.float32)

    in_maps = []
    for i in range(NCORES):
        in_maps.append(
            {
                "dense": dense[BS * i : BS * (i + 1)],
                "vrep": vrep,
                "crep": crep,
                "wlrep": wlrep,
                "cst": cst,
            }
        )
    return in_maps


def kernel(**inputs) -> np.ndarray:
    from concourse.bass_utils import run_bass_kernel_spmd

    nc = _get_program()
    in_maps = _host_prep(inputs)
    res = run_bass_kernel_spmd(nc, in_maps, core_ids=list(range(NCORES)))
    outs = [np.asarray(res.results[i]["out"], np.float32) for i in range(NCORES)]
    return np.concatenate(outs).reshape(B, 1)


# revision 3
# speedup vs baseline: 1.2306x; 1.2306x over previous
"""Trainium2 Bass kernel for nn_AFM (attentional factorization machine).

Mathematical reduction used (validated against the reference):
  - softmax over a size-1 axis == 1, so the attention MLP is dead code and
    fAtt = mean(fPI, axis=1).
  - FM identity per (b, m): sum_{i<j} x_i x_j = ((sum_i x_i)^2 - sum_i x_i^2)/2
    with x_i = dense[b,i,m] * v[i,m].
  So with S1[b,m] = sum_n dense[b,n,m] v[n,m], S2[b,m] = sum_n (dense[b,n,m] v[n,m])^2,
  c[m] = Wp[m] / (2 * P):
    out[b] = sum_n dense[b,n,0] Wl[n] + bl + bp + sum_m c[m] (S1[b,m]^2 - S2[b,m])

Sharding: pure data parallel, batch 4096 -> 512 rows on each of 8 cores.

Kernel structure per 128-row tile (natural [b, (n,m)] layout):
  - cast-load dense to bf16 (SWDGE DMA cast)
  - dv = d * v          (DVE, bf16 2x)
  - S1 = sum_n dv       (DVE, log-tree of contiguous adds over the n-major axis)
  - sq = dv^2           (ACT square)
  - S2 = sum_n sq       (GpSimd, log-tree)
  - combine with chained custom-DVE fused multiply-reduces
"""

import numpy as np

B, N, M = 4096, 32, 64
NM = N * M                  # 2048
NCORES = 8
BS = B // NCORES            # 512 rows per core
TILES = BS // 128           # 4 tiles of 128 batch rows per core
P_PAIRS = N * (N - 1) // 2  # 496

_CACHE = {}


def _tree_reduce(nc, engine, pool, src, dtype_mid, dtype_out, tag):
    """Sum over the n axis of a [128, N*M] n-major tile via contiguous
    halving adds. Returns a [128, M] tile of dtype_out."""
    cur = src
    width = NM
    lvl = 0
    while width > M:
        half = width // 2
        out_dt = dtype_out if half == M else dtype_mid
        nxt = pool.tile([128, half], out_dt, tag=f"{tag}_l{lvl}")
        engine.tensor_add(nxt[:], cur[:, 0:half], cur[:, half:width])
        cur = nxt
        width = half
        lvl += 1
    return cur


def _build_program():
    import concourse.tile as tile
    from concourse import bacc, mybir
    from concourse.dve_ops import TENSOR_TENSOR_REDUCE as CTTR

    f32 = mybir.dt.float32
    bf16 = mybir.dt.bfloat16
    alu = mybir.AluOpType

    nc = bacc.Bacc("TRN2", target_bir_lowering=False, debug=False)
    dense = nc.dram_tensor("dense", [BS, NM], f32, kind="ExternalInput").ap()
    vrep = nc.dram_tensor("vrep", [128, NM], bf16, kind="ExternalInput").ap()
    crep = nc.dram_tensor("crep", [128, M], f32, kind="ExternalInput").ap()
    wlrep = nc.dram_tensor("wlrep", [128, N], f32, kind="ExternalInput").ap()
    cst = nc.dram_tensor("cst", [128, 1], f32, kind="ExternalInput").ap()
    out = nc.dram_tensor("out", [BS], f32, kind="ExternalOutput").ap()

    with tile.TileContext(nc) as tc:
        with (
            tc.tile_pool(name="params", bufs=1) as ppool,
            tc.tile_pool(name="data", bufs=3) as dpool,
            tc.tile_pool(name="work", bufs=3) as wpool,
            tc.tile_pool(name="tree", bufs=3) as tpool,
            tc.tile_pool(name="small", bufs=3) as spool,
        ):
            vrep_t = ppool.tile([128, NM], bf16, tag="vrep")
            nc.sync.dma_start(vrep_t[:], vrep[:, :])
            crep_t = ppool.tile([128, M], f32, tag="crep")
            nc.sync.dma_start(crep_t[:], crep[:, :])
            wlrep_t = ppool.tile([128, N], f32, tag="wlrep")
            nc.sync.dma_start(wlrep_t[:], wlrep[:, :])
            cst_t = ppool.tile([128, 1], f32, tag="cst")
            nc.sync.dma_start(cst_t[:], cst[:, :])

            for t in range(TILES):
                d = dpool.tile([128, NM], bf16, tag="d")
                nc.gpsimd.dma_start(d[:], dense[128 * t : 128 * (t + 1), :])

                dv = wpool.tile([128, NM], bf16, tag="dv")
                nc.vector.tensor_mul(dv[:], d[:], vrep_t[:])

                s1 = _tree_reduce(nc, nc.vector, tpool, dv, bf16, f32, "s1")

                sq = wpool.tile([128, NM], bf16, tag="sq")
                nc.scalar.square(sq[:], dv[:])
                s2 = _tree_reduce(nc, nc.gpsimd, tpool, sq, bf16, f32, "s2")

                # chained fused reduces (custom-DVE TTR: accum = s0 + sum(in0*in1*s1)):
                #   pc1 = (bl+bp) + sum_m c*S1^2
                #   pc2 = pc1 - sum_m c*S2
                #   o2  = pc2 + sum_n dense[:,n,0]*Wl[n]
                cs1 = spool.tile([128, M], f32, tag="cs1")
                nc.vector.tensor_mul(cs1[:], s1[:], crep_t[:])
                junk_a = spool.tile([128, M], f32, tag="junk_a")
                pc1 = spool.tile([128, 1], f32, tag="pc1")
                nc.vector._custom_dve(
                    CTTR, out=junk_a[:], in0=cs1[:], in1=s1[:],
                    s0=cst_t[:], s1=1.0, accum_out=pc1[:],
                )
                junk_b = spool.tile([128, M], f32, tag="junk_b")
                pc2 = spool.tile([128, 1], f32, tag="pc2")
                nc.vector._custom_dve(
                    CTTR, out=junk_b[:], in0=s2[:], in1=crep_t[:],
                    s0=pc1[:], s1=-1.0, accum_out=pc2[:],
                )

                d_col0 = (
                    d[:]
                    .rearrange("p (n m) -> p n m", n=N)[:, :, 0:1]
                    .rearrange("p n one -> p (n one)")
                )
                junk_c = spool.tile([128, N], f32, tag="junk_c")
                o2 = spool.tile([128, 1], f32, tag="o2")
                nc.vector._custom_dve(
                    CTTR, out=junk_c[:], in0=d_col0, in1=wlrep_t[:],
                    s0=pc2[:], s1=1.0, accum_out=o2[:],
                )

                nc.sync.dma_start(out[128 * t : 128 * (t + 1)], o2[:])

    nc.compile()
    return nc


def _get_program():
    if "nc" not in _CACHE:
        _CACHE["nc"] = _build_program()
    return _CACHE["nc"]


def _host_prep(inputs):
    import ml_dtypes

    dense = np.ascontiguousarray(
        np.asarray(inputs["dense"], dtype=np.float32).reshape(B, NM)
    )
    v = np.asarray(inputs["v"], dtype=np.float32).reshape(1, NM)
    Wl = np.asarray(inputs["Wl"], dtype=np.float32).reshape(N)
    Wp = np.asarray(inputs["Wp"], dtype=np.float32).reshape(M)
    bl = float(np.asarray(inputs["bl"], dtype=np.float32).reshape(-1)[0])
    bp = float(np.asarray(inputs["bp"], dtype=np.float32).reshape(-1)[0])

    c = (Wp / (2.0 * P_PAIRS)).astype(np.float32)
    vrep = np.ascontiguousarray(
        np.broadcast_to(v.astype(ml_dtypes.bfloat16), (128, NM))
    )
    crep = np.ascontiguousarray(np.broadcast_to(c[None, :], (128, M)))
    wlrep = np.ascontiguousarray(np.broadcast_to(Wl[None, :], (128, N)))
    cst = np.full((128, 1), bl + bp, dtype=np.float32)

    in_maps = []
    for i in range(NCORES):
        in_maps.append(
            {
                "dense": dense[BS * i : BS * (i + 1)],
                "vrep": vrep,
                "crep": crep,
                "wlrep": wlrep,
                "cst": cst,
            }
        )
    return in_maps


def kernel(**inputs) -> np.ndarray:
    from concourse.bass_utils import run_bass_kernel_spmd

    nc = _get_program()
    in_maps = _host_prep(inputs)
    res = run_bass_kernel_spmd(nc, in_maps, core_ids=list(range(NCORES)))
    outs = [np.asarray(res.results[i]["out"], np.float32) for i in range(NCORES)]
    return np.concatenate(outs).reshape(B, 1)
